# revision 24
# baseline (speedup 1.0000x reference)
"""Trainium2 Bass/Tile kernel for a pre-norm causal decoder block.

Math (matches the jax reference):
    h   = LN1(x) * g1 + beta1
    q,k,v = per-head projections of h (D_HEAD=21, 6 heads)
    sT  = (k @ q^T) / sqrt(21) + causal mask        (scores, transposed)
    e   = exp(sT)                                   (no max-subtraction; scores are tiny)
    o   = (e^T @ [v | 1]) -> per-(t,head) denominator in the appended column
    att = (o / denom) @ Wo + bo
    x1  = x + att
    out = x1 + relu(LN2(x1)*g2+beta2 @ W1 + b1) @ W2 + b2

Sharding: pure data parallelism, batch 512 -> 64 per core across 8 cores.

Layout strategy (per core):
  - tokens T=128 occupy SBUF partitions for LN/residual phases
  - hh is transposed on the PE so q/k/v projections contract over d
  - qT/kT are stored head-padded to 32 partitions (4 heads in "A" [128,*],
    2 heads in "B" [64,*]) so score matmuls are K=32 row-tiles
  - scores are computed transposed (sT[s,t]) so the softmax denominator is
    a matmul-accumulated ones-column and no attention transpose is needed
  - causal mask is added in-PSUM via an identity matmul (values -30 => exp ~ 1e-13)
  - all matmul operands bf16, PSUM accumulation fp32, LN/softmax arithmetic fp32

Engine-balance notes (perfetto-informed):
  - LN mean/var via one bn_stats per group + bn_aggr per batch (DVE)
  - rstd = Exp(-0.5 * Log(var + eps)) so the ONLY ACT table set used in the
    whole kernel is natural_log_exp (exp/log/relu/copy); Sqrt would force a
    ~2.7us table reload per phase transition
  - LN apply ((x - mean) * rstd) on the vector engine via per-partition PTR
    scalars; gpsimd tensor_scalar measures ~2.1us per [128,128] tile (17x DVE)
  - PSUM->SBUF evacuations and elementwise tails batched over pairs of
    batches to amortize the fixed ~60-125 cycle per-op DVE/ACT overhead
"""

import os
import numpy as np
import ml_dtypes

from contextlib import ExitStack

import concourse.bass as bass
import concourse.bacc as bacc
import concourse.tile as tile
from concourse import mybir
from concourse.bass_utils import run_bass_kernel_spmd

# Pin every activation to the one table set that contains all functions this
# kernel uses (exp, ln, relu, copy/identity). The default chooser picks the
# first set per function, which alternates exp_and_others <-> natural_log and
# costs a ~1.5us ACT_TABLE_LOAD + drain per switch, ~44 times per core run --
# and the resulting scalar-engine stalls starve the PE long enough to drop its
# HAM clock from 2.4 to 1.2 GHz once per group. Blanking the other sets (while
# keeping list order, which is what act_func_set_id indexes) forces a single
# load at kernel start.
_ONE_ACT_SET = "natural_log_exp_and_others"
_orig_get_activation_tables = bacc.get_activation_tables


def _single_set_activation_tables(arch):
    tabs = _orig_get_activation_tables(arch)
    return {
        name: (funcs if name == _ONE_ACT_SET else set())
        for name, funcs in tabs.items()
    }


bacc.get_activation_tables = _single_set_activation_tables

BF = mybir.dt.bfloat16
F32 = mybir.dt.float32
NPBF = ml_dtypes.bfloat16

B, T, D = 512, 128, 128
NH, DH = 6, 21
DC = NH * DH  # 126
DFF = 512
NCORES = 8
BPC = B // NCORES  # 64 batches per core
G = 4              # batches per group (free-dim batching of qkv projections)
EPS = 1e-5
MASK_NEG = -30.0
SM_SCALE = 1.0 / np.sqrt(np.float32(DH))

AF = mybir.ActivationFunctionType
ALU = mybir.AluOpType
AX = mybir.AxisListType


def _bf(a):
    return np.ascontiguousarray(np.asarray(a, dtype=np.float32)).astype(NPBF)


def _prep_weights(Wq, Wk, Wv, Wo, bo, W1, b1, W2, b2, g1, beta1, g2, beta2):
    """Host-side folding/packing. Returns dict of named arrays + flags."""
    Wq = np.asarray(Wq, np.float64)
    Wk = np.asarray(Wk, np.float64)
    Wv = np.asarray(Wv, np.float64)
    g1 = np.asarray(g1, np.float64)
    g2 = np.asarray(g2, np.float64)
    beta1 = np.asarray(beta1, np.float64)
    beta2 = np.asarray(beta2, np.float64)
    W1 = np.asarray(W1, np.float64)

    # fold g1 into the qkv projections, 1/sqrt(DH) into Wq
    Wq_f = g1[None, :, None] * Wq * SM_SCALE   # [h, d, e]
    Wk_f = g1[None, :, None] * Wk
    Wv_f = g1[None, :, None] * Wv

    # pair-packed q/k: tensor i holds heads 2i (rows 0..20) and 2i+1 (rows 32..52)
    # 64-partition tensors limit PE row-tile concurrency to 2, and adjacent
    # heads' score matmuls alternate PSUM banks (same-bank concurrent PE
    # writes crash the device)
    wq_p = np.zeros((3, D, 64), np.float64)
    wk_p = np.zeros((3, D, 64), np.float64)
    for h in range(NH):
        i, off = h // 2, 32 * (h % 2)
        wq_p[i, :, off:off + DH] = Wq_f[h]
        wk_p[i, :, off:off + DH] = Wk_f[h]
    wv = np.concatenate([Wv_f[h] for h in range(NH)], axis=1)  # [128, 126]

    # beta1 contributions (rank-1 into qT/kT/v)
    qb = np.einsum("d,hde->he", beta1, Wq) * SM_SCALE   # [6, 21]
    kb = np.einsum("d,hde->he", beta1, Wk)
    vb = np.einsum("d,hde->he", beta1, Wv)
    qb_p = np.zeros((64, 3), np.float64)
    kb_p = np.zeros((64, 3), np.float64)
    for h in range(NH):
        i, off = h // 2, 32 * (h % 2)
        qb_p[off:off + DH, i] = qb[h]
        kb_p[off:off + DH, i] = kb[h]
    vb_r = vb.reshape(1, DC)

    w1 = g2[:, None] * W1                     # [128, 512]
    b1_eff = np.asarray(b1, np.float64) + beta2 @ W1   # [512]
    w2 = np.asarray(W2, np.float64).reshape(4, 128, D).transpose(1, 0, 2)  # [128,4,128]

    # additive causal mask (pre-exp): 0 where s <= t else -30
    mask1 = np.where(np.arange(T)[:, None] <= np.arange(T)[None, :], 0.0, MASK_NEG)
    mask3 = np.tile(mask1, (1, 3))            # [T, 384] for one score bank

    out = {
        "wq_p": _bf(wq_p), "wk_p": _bf(wk_p),
        "wv": _bf(wv),
        "wo": _bf(Wo), "w1": _bf(w1), "w2": _bf(w2),
        "mask3": _bf(mask3),
        "ident": _bf(np.eye(128)),
        "qb_p": np.asarray(qb_p, np.float32),
        "kb_p": np.asarray(kb_p, np.float32),
        "vb_r": np.asarray(vb_r, np.float32),
        "bo_r": np.asarray(bo, np.float32).reshape(1, D),
        "b2_r": np.asarray(b2, np.float32).reshape(1, D),
        "b1e": np.ascontiguousarray(
            np.asarray(b1_eff, np.float64).reshape(4, 128).T, dtype=np.float32
        ),  # [128, 4] per-partition relu bias per chunk
    }
    flags = {
        "qkv_bias": bool(np.any(beta1 != 0.0)),
        "bo": bool(np.any(np.asarray(bo) != 0.0)),
        "b2": bool(np.any(np.asarray(b2) != 0.0)),
        "b1": bool(np.any(out["b1e"] != 0.0)),
    }
    return out, flags


def _emit(ctx, tc, aps, flags, bpc):
    nc = tc.nc
    x_ap = aps["x"]
    y_ap = aps["y"]

    singles = ctx.enter_context(tc.tile_pool(name="singles", bufs=1))
    sb_g = ctx.enter_context(tc.tile_pool(name="sb_g", bufs=4))
    sb_b = ctx.enter_context(tc.tile_pool(name="sb_b", bufs=6))
    sb_s = ctx.enter_context(tc.tile_pool(name="sb_s", bufs=12))
    psg = ctx.enter_context(tc.tile_pool(name="psg", bufs=2, space="PSUM"))
    pss = ctx.enter_context(tc.tile_pool(name="pss", bufs=2, space="PSUM"))
    psb = ctx.enter_context(tc.tile_pool(name="psb", bufs=1, space="PSUM"))
    psf = ctx.enter_context(tc.tile_pool(name="psf", bufs=1, space="PSUM"))

    # ---- resident constants -------------------------------------------------
    def load_const(name, shape, dtype=BF):
        t = singles.tile(list(shape), dtype, tag=name, name=name)
        nc.sync.dma_start(out=t[:], in_=aps[name])
        return t

    wq_p = singles.tile([D, 3, 64], BF, tag="wq_p")
    nc.sync.dma_start(out=wq_p[:], in_=aps["wq_p"].rearrange("i d e -> d i e"))
    wk_p = singles.tile([D, 3, 64], BF, tag="wk_p")
    nc.sync.dma_start(out=wk_p[:], in_=aps["wk_p"].rearrange("i d e -> d i e"))
    wv = load_const("wv", [D, DC])
    wo = load_const("wo", [DC, D])
    w1 = load_const("w1", [D, DFF])
    w2 = load_const("w2", [D, 4, D])
    mask3 = load_const("mask3", [T, 384])
    ident = load_const("ident", [128, 128])
    if flags["qkv_bias"]:
        qb_p = load_const("qb_p", [64, 3], F32)
        kb_p = load_const("kb_p", [64, 3], F32)
        vb_rep = singles.tile([128, DC], F32, tag="vb_rep")
        nc.sync.dma_start(out=vb_rep[:], in_=aps["vb_r"].to_broadcast([128, DC]))
    if flags["bo"]:
        bo_rep = singles.tile([128, D], F32, tag="bo_rep")
        nc.sync.dma_start(out=bo_rep[:], in_=aps["bo_r"].to_broadcast([128, D]))
    if flags["b2"]:
        b2_rep = singles.tile([128, D], F32, tag="b2_rep")
        nc.sync.dma_start(out=b2_rep[:], in_=aps["b2_r"].to_broadcast([128, D]))
    if flags["b1"]:
        b1e = load_const("b1e", [128, 4], F32)

    eps_t = singles.tile([128, 1], F32, tag="eps")
    nc.vector.memset(eps_t[:], EPS)

    n_groups = bpc // G
    repeat = int(os.environ.get("K_REPEAT", "1"))
    glist = [gg for _ in range(repeat) for gg in range(n_groups)]

    def load_x(g):
        x_t = sb_g.tile([T, G, D], F32, tag="x_t")
        nc.sync.dma_start(
            out=x_t[:], in_=x_ap[g * G:(g + 1) * G].rearrange("b t d -> t b d")
        )
        return x_t

    def ln_stats(x_t, tag):
        """mean + rstd for G batches: bn_stats/bn_aggr on DVE, then
        rstd = Exp(-0.5*Log(var+eps)) on ACT (stays in the exp/log table set)."""
        st = sb_s.tile([128, G, 6], F32, tag=f"st{tag}")
        aggr = sb_s.tile([128, G, 2], F32, tag=f"aggr{tag}")
        for b in range(G):
            nc.vector.bn_stats(out=st[:, b, :], in_=x_t[:, b, :])
            nc.vector.bn_aggr(out=aggr[:, b, :], in_=st[:, b, :])
        logv = sb_s.tile([128, G], F32, tag=f"logv{tag}")
        nc.scalar.activation(
            out=logv[:], in_=aggr[:, :, 1], func=AF.Ln, bias=eps_t[:], scale=1.0,
        )
        rstd = sb_s.tile([128, G], F32, tag=f"rstd{tag}")
        nc.scalar.activation(
            out=rstd[:], in_=logv[:], func=AF.Exp, bias=0.0, scale=-0.5,
        )
        return aggr, rstd

    def ln_apply(out_ap, in_ap, aggr, rstd, b):
        nc.vector.tensor_scalar(
            out=out_ap, in0=in_ap,
            scalar1=aggr[:, b, 0:1], scalar2=rstd[:, b:b + 1],
            op0=ALU.subtract, op1=ALU.mult,
        )

    def phase_A(x_t):
        """LN1 + transpose + q/k/v projections for one group."""
        aggr1, rstd1 = ln_stats(x_t, "1")
        # LN1 apply on ACT (DVE is the busiest engine): (x-m)*r = r*x + (-m*r)
        rneg1 = sb_s.tile([128, G], F32, tag="rneg1")
        nc.vector.tensor_scalar_mul(out=rneg1[:], in0=rstd1[:], scalar1=-1.0)
        nb1 = sb_s.tile([128, G], F32, tag="nb1")
        nc.vector.tensor_tensor(
            out=nb1[:], in0=aggr1[:, :, 0], in1=rneg1[:], op=ALU.mult,
        )

        hh = sb_g.tile([T, G, D], BF, tag="hh")
        for b in range(G):
            nc.scalar.activation(
                out=hh[:, b, :], in_=x_t[:, b, :], func=AF.Identity,
                scale=rstd1[:, b:b + 1], bias=nb1[:, b:b + 1],
            )

        # 4 transposes into one PSUM tile, one batched evacuation
        tp = psg.tile([D, G, T], BF, tag="gp")
        for b in range(G):
            nc.tensor.transpose(
                out=tp[:, b, :], in_=hh[:, b, :], identity=ident[:],
            )
        hhT = sb_g.tile([D, G, T], BF, tag="hhT")
        nc.vector.tensor_copy(
            out=hhT[:].rearrange("d b t -> d (b t)"),
            in_=tp[:].rearrange("d b t -> d (b t)"),
        )

        def proj(w_slice):
            ps = psg.tile([64, G * T], F32, tag="gp")
            nc.tensor.matmul(
                ps[:], w_slice, hhT[:].rearrange("d b t -> d (b t)"),
                start=True, stop=True,
            )
            return ps

        qt = sb_g.tile([64, 3, G, T], BF, tag="qt")
        kt = sb_g.tile([64, 3, G, T], BF, tag="kt")
        for i in range(3):
            q_ps = proj(wq_p[:, i, :])
            if flags["qkv_bias"]:
                nc.scalar.activation(
                    out=qt[:, i, :, :].rearrange("p b t -> p (b t)"), in_=q_ps[:],
                    func=AF.Identity, bias=qb_p[:, i:i + 1], scale=1.0,
                )
            else:
                nc.scalar.copy(
                    out=qt[:, i, :, :].rearrange("p b t -> p (b t)"), in_=q_ps[:]
                )
            k_ps = proj(wk_p[:, i, :])
            if flags["qkv_bias"]:
                nc.vector.tensor_scalar_add(
                    out=kt[:, i, :, :].rearrange("p b t -> p (b t)"), in0=k_ps[:],
                    scalar1=kb_p[:, i:i + 1],
                )
            else:
                nc.vector.tensor_copy(
                    out=kt[:, i, :, :].rearrange("p b t -> p (b t)"), in_=k_ps[:]
                )

        v_ps = psg.tile([T, G, NH, DH], F32, tag="gp")
        for b in range(G):
            nc.tensor.matmul(
                v_ps[:, b, :, :], hhT[:, b, :], wv[:],
                start=True, stop=True, skip_group_check=True,
            )
        v_sb = sb_g.tile([T, G, NH, DH + 1], BF, tag="v_sb")
        if flags["qkv_bias"]:
            vb3 = vb_rep[:].rearrange("p (h e) -> p h e", h=NH)
            vb4 = bass.AP(
                tensor=vb3.tensor, offset=vb3.offset,
                ap=[vb3.ap[0], [0, G], vb3.ap[1], vb3.ap[2]],
            )
            nc.vector.tensor_tensor(
                out=v_sb[:, :, :, 0:DH], in0=v_ps[:], in1=vb4, op=ALU.add,
            )
        else:
            nc.vector.tensor_copy(out=v_sb[:, :, :, 0:DH], in_=v_ps[:])
        nc.gpsimd.memset(v_sb[:, :, :, DH:DH + 1], 1.0)
        return qt, kt, v_sb

    def phase_B(x_t, qt, kt, v_sb):
        """Causal attention + residual, processed in pairs of batches."""
        x1_all = sb_b.tile([T, G, D], F32, tag="x1")
        for p in range(G // 2):
            eTs = []
            for bb in range(2):
                b = 2 * p + bb
                # one 2-bank PSUM tile per batch: bank 0 cols 0:384, bank 1
                # cols 512:896 (a matmul output may not straddle banks)
                s2 = pss.tile([T, 2, 512], F32, tag="sT")
                # mask lands first (start=True over the whole bank); score
                # matmuls then accumulate onto it, one per 128-col region
                for bank in range(2):
                    nc.tensor.matmul(
                        s2[:, bank, 0:384], ident[:], mask3[:],
                        start=True, stop=False, skip_group_check=True,
                    )
                for h in range(NH):
                    i, off = h // 2, 32 * (h % 2)
                    nc.tensor.matmul(
                        s2[:, h % 2, 128 * (h // 2):128 * (h // 2 + 1)],
                        kt[off:off + 32, i, b, :],
                        qt[off:off + 32, i, b, :],
                        start=False, stop=(h >= 4), skip_group_check=True,
                    )
                eT = sb_b.tile([T, 2, 384], BF, tag="eT")
                nc.scalar.activation(out=eT[:], in_=s2[:, :, 0:384], func=AF.Exp)
                eTs.append(eT[:].rearrange("t b c -> t (b c)"))

            # o[t, (b, h, e+1)] with the softmax denominator in the last column
            o_ps = psb.tile([T, 2, NH, DH + 1], F32, tag="bp")
            for bb in range(2):
                b = 2 * p + bb
                for h in range(NH):
                    ecol = 384 * (h % 2) + 128 * (h // 2)
                    nc.tensor.matmul(
                        o_ps[:, bb, h, :],
                        eTs[bb][:, ecol:ecol + 128],
                        v_sb[:, b, h, :],
                        start=True, stop=True, skip_group_check=True,
                    )

            recip = sb_s.tile([128, 2, NH, 1], F32, tag="recip")
            nc.vector.reciprocal(out=recip[:], in_=o_ps[:, :, :, DH:DH + 1])
            o_sb = sb_b.tile([T, 2, NH, DH], BF, tag="o_sb")
            nc.vector.tensor_tensor(
                out=o_sb[:], in0=o_ps[:, :, :, 0:DH],
                in1=recip[:].to_broadcast([128, 2, NH, DH]), op=ALU.mult,
            )

            oT_ps = psb.tile([DC, 2, T], BF, tag="bp")
            for bb in range(2):
                nc.tensor.transpose(
                    out=oT_ps[:, bb, :],
                    in_=o_sb[:, bb, :, :].rearrange("t h e -> t (h e)"),
                    identity=ident[:],
                )
            oT_sb = sb_b.tile([DC, 2, T], BF, tag="oT")
            nc.vector.tensor_copy(
                out=oT_sb[:].rearrange("p b t -> p (b t)"),
                in_=oT_ps[:].rearrange("p b t -> p (b t)"),
            )

            att_ps = psb.tile([T, 2, D], F32, tag="bp")
            for bb in range(2):
                nc.tensor.matmul(
                    att_ps[:, bb, :], oT_sb[:, bb, :], wo[:],
                    start=True, stop=True, skip_group_check=True,
                )

            nc.vector.tensor_tensor(
                out=x1_all[:, 2 * p:2 * p + 2, :],
                in0=x_t[:, 2 * p:2 * p + 2, :], in1=att_ps[:], op=ALU.add,
            )
            if flags["bo"]:
                bo4 = bass.AP(
                    tensor=bo_rep.tensor, offset=bo_rep[:].offset,
                    ap=[bo_rep[:].ap[0], [0, 2], bo_rep[:].ap[1]],
                )
                nc.vector.tensor_tensor(
                    out=x1_all[:, 2 * p:2 * p + 2, :],
                    in0=x1_all[:, 2 * p:2 * p + 2, :], in1=bo4, op=ALU.add,
                )
        return x1_all

    def phase_C(g, x1_all):
        """LN2 + feed-forward + residual + store, in pairs of batches."""
        aggr2, rstd2 = ln_stats(x1_all, "2")

        for p in range(G // 2):
            hh2 = sb_b.tile([T, 2, D], BF, tag="hh2")
            for bb in range(2):
                b = 2 * p + bb
                ln_apply(hh2[:, bb, :], x1_all[:, b, :], aggr2, rstd2, b)

            h2T_ps = psf.tile([D, 2, T], BF, tag="fp")
            for bb in range(2):
                nc.tensor.transpose(
                    out=h2T_ps[:, bb, :], in_=hh2[:, bb, :], identity=ident[:],
                )
            h2T = sb_b.tile([D, 2, T], BF, tag="h2T")
            nc.vector.tensor_copy(
                out=h2T[:].rearrange("d b t -> d (b t)"),
                in_=h2T_ps[:].rearrange("d b t -> d (b t)"),
            )

            # ff1 over the batch pair: one stationary load per w1 chunk,
            # N=256 moving; two chunks share one PSUM bank
            r_sb = sb_b.tile([128, 2, 4, T], BF, tag="r_sb")
            for cc in range(2):
                ff1_ps = psf.tile([128, 2, 2, T], F32, tag="fp")
                for c2 in range(2):
                    c = 2 * cc + c2
                    nc.tensor.matmul(
                        ff1_ps[:, c2, :, :].rearrange("p b t -> p (b t)"),
                        w1[:, 128 * c:128 * (c + 1)],
                        h2T[:].rearrange("d b t -> d (b t)"),
                        start=True, stop=True, skip_group_check=True,
                    )
                if flags["b1"]:
                    for c2 in range(2):
                        c = 2 * cc + c2
                        for bb in range(2):
                            nc.scalar.activation(
                                out=r_sb[:, bb, c, :], in_=ff1_ps[:, c2, bb, :],
                                func=AF.Relu, bias=b1e[:, c:c + 1], scale=1.0,
                            )
                else:
                    nc.scalar.activation(
                        out=r_sb[:, :, 2 * cc:2 * cc + 2, :].rearrange(
                            "p b c t -> p c b t"
                        ),
                        in_=ff1_ps[:],
                        func=AF.Relu,
                    )

            ff2_ps = psf.tile([T, 2, D], F32, tag="fp")
            for bb in range(2):
                for c in range(4):
                    nc.tensor.matmul(
                        ff2_ps[:, bb, :], r_sb[:, bb, c, :], w2[:, c, :],
                        start=(c == 0), stop=(c == 3), skip_group_check=True,
                    )

            out_sb = sb_b.tile([T, 2, D], F32, tag="out_sb")
            nc.vector.tensor_tensor(
                out=out_sb[:], in0=x1_all[:, 2 * p:2 * p + 2, :], in1=ff2_ps[:],
                op=ALU.add,
            )
            if flags["b2"]:
                b24 = bass.AP(
                    tensor=b2_rep.tensor, offset=b2_rep[:].offset,
                    ap=[b2_rep[:].ap[0], [0, 2], b2_rep[:].ap[1]],
                )
                nc.vector.tensor_tensor(
                    out=out_sb[:], in0=out_sb[:], in1=b24, op=ALU.add,
                )
            a = g * G + 2 * p
            nc.gpsimd.dma_start(
                out=y_ap[a:a + 2].rearrange("b t d -> t b d"), in_=out_sb[:]
            )

    # software pipeline: B(g) | A(g+1) | C(g)
    x_cur = load_x(glist[0])
    A_cur = phase_A(x_cur)
    for gi, g in enumerate(glist):
        more = gi + 1 < len(glist)
        if more:
            x_nxt = load_x(glist[gi + 1])
        x1 = phase_B(x_cur, *A_cur)
        if more:
            A_nxt = phase_A(x_nxt)
        phase_C(g, x1)
        if more:
            x_cur, A_cur = x_nxt, A_nxt


def build_program(weights, flags, bpc=BPC):
    nc = bacc.Bacc("TRN2", target_bir_lowering=False, debug=False)
    aps = {}
    aps["x"] = nc.dram_tensor("x", [bpc, T, D], F32, kind="ExternalInput").ap()
    aps["y"] = nc.dram_tensor("y", [bpc, T, D], F32, kind="ExternalOutput").ap()
    for name, arr in weights.items():
        dt = F32 if arr.dtype == np.float32 else BF
        aps[name] = nc.dram_tensor(name, list(arr.shape), dt, kind="ExternalInput").ap()
    with tile.TileContext(nc) as tc:
        with ExitStack() as ctx:
            _emit(ctx, tc, aps, flags, bpc)
    nc.compile()
    return nc


_CACHE = {}


def _get_program_and_maps(x, args):
    x = np.asarray(x, np.float32)
    weights, flags = _prep_weights(*args)
    key = tuple(sorted(flags.items()))
    if key not in _CACHE:
        _CACHE[key] = build_program(weights, flags)
    nc = _CACHE[key]
    in_maps = []
    for c in range(NCORES):
        m = {"x": np.ascontiguousarray(x[c * BPC:(c + 1) * BPC])}
        m.update(weights)
        in_maps.append(m)
    return nc, in_maps


def kernel(x, Wq, Wk, Wv, Wo, bo, W1, b1, W2, b2, g1, beta1, g2, beta2):
    nc, in_maps = _get_program_and_maps(
        x, (Wq, Wk, Wv, Wo, bo, W1, b1, W2, b2, g1, beta1, g2, beta2)
    )
    res = run_bass_kernel_spmd(nc, in_maps, list(range(NCORES)))
    out = np.concatenate([res.results[c]["y"] for c in range(NCORES)], axis=0)
    return out.astype(np.float32)


def run_traced(inputs):
    """Profiled run; returns BassKernelResults with exec_time_ns if available."""
    args = tuple(
        inputs[k]
        for k in ("Wq", "Wk", "Wv", "Wo", "bo", "W1", "b1", "W2", "b2",
                  "g1", "beta1", "g2", "beta2")
    )
    nc, in_maps = _get_program_and_maps(inputs["x"], args)
    return run_bass_kernel_spmd(nc, in_maps, list(range(NCORES)), trace=True)


# revision 25
# speedup vs baseline: 1.0537x; 1.0537x over previous
"""Trainium2 Bass/Tile kernel for a pre-norm causal decoder block.

Math (matches the jax reference):
    h   = LN1(x) * g1 + beta1
    q,k,v = per-head projections of h (D_HEAD=21, 6 heads)
    sT  = (k @ q^T) / sqrt(21) + causal mask        (scores, transposed)
    e   = exp(sT)                                   (no max-subtraction; scores are tiny)
    o   = (e^T @ [v | 1]) -> per-(t,head) denominator in the appended column
    att = (o / denom) @ Wo + bo
    x1  = x + att
    out = x1 + relu(LN2(x1)*g2+beta2 @ W1 + b1) @ W2 + b2

Sharding: pure data parallelism, batch 512 -> 64 per core across 8 cores.

Layout strategy (per core):
  - tokens T=128 occupy SBUF partitions for LN/residual phases
  - hh is transposed on the PE so q/k/v projections contract over d
  - qT/kT are stored head-padded to 32 partitions (4 heads in "A" [128,*],
    2 heads in "B" [64,*]) so score matmuls are K=32 row-tiles
  - scores are computed transposed (sT[s,t]) so the softmax denominator is
    a matmul-accumulated ones-column and no attention transpose is needed
  - causal mask is added in-PSUM via an identity matmul (values -30 => exp ~ 1e-13)
  - all matmul operands bf16, PSUM accumulation fp32, LN/softmax arithmetic fp32

Engine-balance notes (perfetto-informed):
  - LN mean/var via one bn_stats per group + bn_aggr per batch (DVE)
  - rstd = Exp(-0.5 * Log(var + eps)) so the ONLY ACT table set used in the
    whole kernel is natural_log_exp (exp/log/relu/copy); Sqrt would force a
    ~2.7us table reload per phase transition
  - LN apply ((x - mean) * rstd) on the vector engine via per-partition PTR
    scalars; gpsimd tensor_scalar measures ~2.1us per [128,128] tile (17x DVE)
  - PSUM->SBUF evacuations and elementwise tails batched over pairs of
    batches to amortize the fixed ~60-125 cycle per-op DVE/ACT overhead
"""

import os
import numpy as np
import ml_dtypes

from contextlib import ExitStack

import concourse.bass as bass
import concourse.bacc as bacc
import concourse.tile as tile
from concourse import mybir
from concourse.bass_utils import run_bass_kernel_spmd

# Pin every activation to the one table set that contains all functions this
# kernel uses (exp, ln, relu, copy/identity). The default chooser picks the
# first set per function, which alternates exp_and_others <-> natural_log and
# costs a ~1.5us ACT_TABLE_LOAD + drain per switch, ~44 times per core run --
# and the resulting scalar-engine stalls starve the PE long enough to drop its
# HAM clock from 2.4 to 1.2 GHz once per group. Blanking the other sets (while
# keeping list order, which is what act_func_set_id indexes) forces a single
# load at kernel start.
_ONE_ACT_SET = "natural_log_exp_and_others"
_orig_get_activation_tables = bacc.get_activation_tables


def _single_set_activation_tables(arch):
    tabs = _orig_get_activation_tables(arch)
    return {
        name: (funcs if name == _ONE_ACT_SET else set())
        for name, funcs in tabs.items()
    }


bacc.get_activation_tables = _single_set_activation_tables

BF = mybir.dt.bfloat16
F32 = mybir.dt.float32
NPBF = ml_dtypes.bfloat16

B, T, D = 512, 128, 128
NH, DH = 6, 21
DC = NH * DH  # 126
DFF = 512
NCORES = 8
BPC = B // NCORES  # 64 batches per core
G = 4              # batches per group (free-dim batching of qkv projections)
EPS = 1e-5
MASK_NEG = -30.0
SM_SCALE = 1.0 / np.sqrt(np.float32(DH))

AF = mybir.ActivationFunctionType
ALU = mybir.AluOpType
AX = mybir.AxisListType


def _bf(a):
    return np.ascontiguousarray(np.asarray(a, dtype=np.float32)).astype(NPBF)


def _prep_weights(Wq, Wk, Wv, Wo, bo, W1, b1, W2, b2, g1, beta1, g2, beta2):
    """Host-side folding/packing. Returns dict of named arrays + flags."""
    Wq = np.asarray(Wq, np.float64)
    Wk = np.asarray(Wk, np.float64)
    Wv = np.asarray(Wv, np.float64)
    g1 = np.asarray(g1, np.float64)
    g2 = np.asarray(g2, np.float64)
    beta1 = np.asarray(beta1, np.float64)
    beta2 = np.asarray(beta2, np.float64)
    W1 = np.asarray(W1, np.float64)

    # fold g1 into the qkv projections, 1/sqrt(DH) into Wq
    Wq_f = g1[None, :, None] * Wq * SM_SCALE   # [h, d, e]
    Wk_f = g1[None, :, None] * Wk
    Wv_f = g1[None, :, None] * Wv

    # pair-packed q/k: tensor i holds heads 2i (rows 0..20) and 2i+1 (rows 32..52)
    # 64-partition tensors limit PE row-tile concurrency to 2, and adjacent
    # heads' score matmuls alternate PSUM banks (same-bank concurrent PE
    # writes crash the device)
    wq_p = np.zeros((3, D, 64), np.float64)
    wk_p = np.zeros((3, D, 64), np.float64)
    for h in range(NH):
        i, off = h // 2, 32 * (h % 2)
        wq_p[i, :, off:off + DH] = Wq_f[h]
        wk_p[i, :, off:off + DH] = Wk_f[h]
    wv = np.concatenate([Wv_f[h] for h in range(NH)], axis=1)  # [128, 126]

    # beta1 contributions (rank-1 into qT/kT/v)
    qb = np.einsum("d,hde->he", beta1, Wq) * SM_SCALE   # [6, 21]
    kb = np.einsum("d,hde->he", beta1, Wk)
    vb = np.einsum("d,hde->he", beta1, Wv)
    qb_p = np.zeros((64, 3), np.float64)
    kb_p = np.zeros((64, 3), np.float64)
    for h in range(NH):
        i, off = h // 2, 32 * (h % 2)
        qb_p[off:off + DH, i] = qb[h]
        kb_p[off:off + DH, i] = kb[h]
    vb_r = vb.reshape(1, DC)

    w1 = g2[:, None] * W1                     # [128, 512]
    b1_eff = np.asarray(b1, np.float64) + beta2 @ W1   # [512]
    w2 = np.asarray(W2, np.float64).reshape(4, 128, D).transpose(1, 0, 2)  # [128,4,128]

    # additive causal mask (pre-exp): 0 where s <= t else -30
    mask1 = np.where(np.arange(T)[:, None] <= np.arange(T)[None, :], 0.0, MASK_NEG)
    mask3 = np.tile(mask1, (1, 3))            # [T, 384] for one score bank

    out = {
        "wq_p": _bf(wq_p), "wk_p": _bf(wk_p),
        "wv": _bf(wv),
        "wo": _bf(Wo), "w1": _bf(w1), "w2": _bf(w2),
        "mask3": _bf(mask3),
        "ident": _bf(np.eye(128)),
        "qb_p": np.asarray(qb_p, np.float32),
        "kb_p": np.asarray(kb_p, np.float32),
        "vb_r": np.asarray(vb_r, np.float32),
        "bo_r": np.asarray(bo, np.float32).reshape(1, D),
        "b2_r": np.asarray(b2, np.float32).reshape(1, D),
        "b1e": np.ascontiguousarray(
            np.asarray(b1_eff, np.float64).reshape(4, 128).T, dtype=np.float32
        ),  # [128, 4] per-partition relu bias per chunk
    }
    flags = {
        "qkv_bias": bool(np.any(beta1 != 0.0)),
        "bo": bool(np.any(np.asarray(bo) != 0.0)),
        "b2": bool(np.any(np.asarray(b2) != 0.0)),
        "b1": bool(np.any(out["b1e"] != 0.0)),
    }
    return out, flags


def _emit(ctx, tc, aps, flags, bpc):
    nc = tc.nc
    x_ap = aps["x"]
    y_ap = aps["y"]

    singles = ctx.enter_context(tc.tile_pool(name="singles", bufs=1))
    sb_g = ctx.enter_context(tc.tile_pool(name="sb_g", bufs=4))
    sb_b = ctx.enter_context(tc.tile_pool(name="sb_b", bufs=6))
    sb_s = ctx.enter_context(tc.tile_pool(name="sb_s", bufs=12))
    psg = ctx.enter_context(tc.tile_pool(name="psg", bufs=2, space="PSUM"))
    pss = ctx.enter_context(tc.tile_pool(name="pss", bufs=2, space="PSUM"))
    psb = ctx.enter_context(tc.tile_pool(name="psb", bufs=1, space="PSUM"))
    psf = ctx.enter_context(tc.tile_pool(name="psf", bufs=1, space="PSUM"))

    # ---- resident constants -------------------------------------------------
    def load_const(name, shape, dtype=BF):
        t = singles.tile(list(shape), dtype, tag=name, name=name)
        nc.sync.dma_start(out=t[:], in_=aps[name])
        return t

    wq_p = singles.tile([D, 3, 64], BF, tag="wq_p")
    nc.sync.dma_start(out=wq_p[:], in_=aps["wq_p"].rearrange("i d e -> d i e"))
    wk_p = singles.tile([D, 3, 64], BF, tag="wk_p")
    nc.sync.dma_start(out=wk_p[:], in_=aps["wk_p"].rearrange("i d e -> d i e"))
    wv = load_const("wv", [D, DC])
    wo = load_const("wo", [DC, D])
    w1 = load_const("w1", [D, DFF])
    w2 = load_const("w2", [D, 4, D])
    mask3 = load_const("mask3", [T, 384])
    ident = load_const("ident", [128, 128])
    if flags["qkv_bias"]:
        qb_p = load_const("qb_p", [64, 3], F32)
        kb_p = load_const("kb_p", [64, 3], F32)
        vb_rep = singles.tile([128, DC], F32, tag="vb_rep")
        nc.sync.dma_start(out=vb_rep[:], in_=aps["vb_r"].to_broadcast([128, DC]))
    if flags["bo"]:
        bo_rep = singles.tile([128, D], F32, tag="bo_rep")
        nc.sync.dma_start(out=bo_rep[:], in_=aps["bo_r"].to_broadcast([128, D]))
    if flags["b2"]:
        b2_rep = singles.tile([128, D], F32, tag="b2_rep")
        nc.sync.dma_start(out=b2_rep[:], in_=aps["b2_r"].to_broadcast([128, D]))
    if flags["b1"]:
        b1e = load_const("b1e", [128, 4], F32)

    eps_t = singles.tile([128, 1], F32, tag="eps")
    nc.vector.memset(eps_t[:], EPS)

    n_groups = bpc // G
    repeat = int(os.environ.get("K_REPEAT", "1"))
    glist = [gg for _ in range(repeat) for gg in range(n_groups)]

    def load_x(g):
        x_t = sb_g.tile([T, G, D], F32, tag="x_t")
        nc.sync.dma_start(
            out=x_t[:], in_=x_ap[g * G:(g + 1) * G].rearrange("b t d -> t b d")
        )
        return x_t

    def ln_stats(x_t, tag):
        """mean + rstd for G batches: bn_stats/bn_aggr on DVE, then
        rstd = Exp(-0.5*Log(var+eps)) on ACT (stays in the exp/log table set)."""
        st = sb_s.tile([128, G, 6], F32, tag=f"st{tag}")
        aggr = sb_s.tile([128, G, 2], F32, tag=f"aggr{tag}")
        for b in range(G):
            nc.vector.bn_stats(out=st[:, b, :], in_=x_t[:, b, :])
            nc.vector.bn_aggr(out=aggr[:, b, :], in_=st[:, b, :])
        logv = sb_s.tile([128, G], F32, tag=f"logv{tag}")
        nc.scalar.activation(
            out=logv[:], in_=aggr[:, :, 1], func=AF.Ln, bias=eps_t[:], scale=1.0,
        )
        rstd = sb_s.tile([128, G], F32, tag=f"rstd{tag}")
        nc.scalar.activation(
            out=rstd[:], in_=logv[:], func=AF.Exp, bias=0.0, scale=-0.5,
        )
        return aggr, rstd

    def ln_apply(out_ap, in_ap, aggr, rstd, b):
        nc.vector.tensor_scalar(
            out=out_ap, in0=in_ap,
            scalar1=aggr[:, b, 0:1], scalar2=rstd[:, b:b + 1],
            op0=ALU.subtract, op1=ALU.mult,
        )

    def phase_A(x_t):
        """LN1 + transpose + q/k/v projections for one group."""
        aggr1, rstd1 = ln_stats(x_t, "1")

        hh = sb_g.tile([T, G, D], BF, tag="hh")
        for b in range(G):
            ln_apply(hh[:, b, :], x_t[:, b, :], aggr1, rstd1, b)

        # 4 transposes into one PSUM tile, one batched evacuation
        tp = psg.tile([D, G, T], BF, tag="gp")
        for b in range(G):
            nc.tensor.transpose(
                out=tp[:, b, :], in_=hh[:, b, :], identity=ident[:],
            )
        hhT = sb_g.tile([D, G, T], BF, tag="hhT")
        nc.vector.tensor_copy(
            out=hhT[:].rearrange("d b t -> d (b t)"),
            in_=tp[:].rearrange("d b t -> d (b t)"),
        )

        def proj(w_slice):
            ps = psg.tile([64, G * T], F32, tag="gp")
            nc.tensor.matmul(
                ps[:], w_slice, hhT[:].rearrange("d b t -> d (b t)"),
                start=True, stop=True,
            )
            return ps

        qt = sb_g.tile([64, 3, G, T], BF, tag="qt")
        kt = sb_g.tile([64, 3, G, T], BF, tag="kt")
        for i in range(3):
            q_ps = proj(wq_p[:, i, :])
            if flags["qkv_bias"]:
                nc.scalar.activation(
                    out=qt[:, i, :, :].rearrange("p b t -> p (b t)"), in_=q_ps[:],
                    func=AF.Identity, bias=qb_p[:, i:i + 1], scale=1.0,
                )
            else:
                nc.scalar.copy(
                    out=qt[:, i, :, :].rearrange("p b t -> p (b t)"), in_=q_ps[:]
                )
            k_ps = proj(wk_p[:, i, :])
            if flags["qkv_bias"]:
                nc.vector.tensor_scalar_add(
                    out=kt[:, i, :, :].rearrange("p b t -> p (b t)"), in0=k_ps[:],
                    scalar1=kb_p[:, i:i + 1],
                )
            else:
                nc.vector.tensor_copy(
                    out=kt[:, i, :, :].rearrange("p b t -> p (b t)"), in_=k_ps[:]
                )

        v_ps = psg.tile([T, G, NH, DH], F32, tag="gp")
        for b in range(G):
            nc.tensor.matmul(
                v_ps[:, b, :, :], hhT[:, b, :], wv[:],
                start=True, stop=True, skip_group_check=True,
            )
        v_sb = sb_g.tile([T, G, NH, DH + 1], BF, tag="v_sb")
        if flags["qkv_bias"]:
            vb3 = vb_rep[:].rearrange("p (h e) -> p h e", h=NH)
            vb4 = bass.AP(
                tensor=vb3.tensor, offset=vb3.offset,
                ap=[vb3.ap[0], [0, G], vb3.ap[1], vb3.ap[2]],
            )
            nc.vector.tensor_tensor(
                out=v_sb[:, :, :, 0:DH], in0=v_ps[:], in1=vb4, op=ALU.add,
            )
        else:
            nc.vector.tensor_copy(out=v_sb[:, :, :, 0:DH], in_=v_ps[:])
        nc.gpsimd.memset(v_sb[:, :, :, DH:DH + 1], 1.0)
        return qt, kt, v_sb

    def phase_B(x_t, qt, kt, v_sb):
        """Causal attention + residual, processed in pairs of batches."""
        x1_all = sb_b.tile([T, G, D], F32, tag="x1")
        for p in range(G // 2):
            eTs = []
            for bb in range(2):
                b = 2 * p + bb
                # one 2-bank PSUM tile per batch: bank 0 cols 0:384, bank 1
                # cols 512:896 (a matmul output may not straddle banks)
                s2 = pss.tile([T, 2, 512], F32, tag="sT")
                # mask lands first (start=True over the whole bank); score
                # matmuls then accumulate onto it, one per 128-col region
                for bank in range(2):
                    nc.tensor.matmul(
                        s2[:, bank, 0:384], ident[:], mask3[:],
                        start=True, stop=False, skip_group_check=True,
                    )
                for h in range(NH):
                    i, off = h // 2, 32 * (h % 2)
                    nc.tensor.matmul(
                        s2[:, h % 2, 128 * (h // 2):128 * (h // 2 + 1)],
                        kt[off:off + 32, i, b, :],
                        qt[off:off + 32, i, b, :],
                        start=False, stop=(h >= 4), skip_group_check=True,
                    )
                eT = sb_b.tile([T, 2, 384], BF, tag="eT")
                nc.scalar.activation(out=eT[:], in_=s2[:, :, 0:384], func=AF.Exp)
                eTs.append(eT[:].rearrange("t b c -> t (b c)"))

            # o[t, (b, h, e+1)] with the softmax denominator in the last column
            o_ps = psb.tile([T, 2, NH, DH + 1], F32, tag="bp")
            for bb in range(2):
                b = 2 * p + bb
                for h in range(NH):
                    ecol = 384 * (h % 2) + 128 * (h // 2)
                    nc.tensor.matmul(
                        o_ps[:, bb, h, :],
                        eTs[bb][:, ecol:ecol + 128],
                        v_sb[:, b, h, :],
                        start=True, stop=True, skip_group_check=True,
                    )

            recip = sb_s.tile([128, 2, NH, 1], F32, tag="recip")
            nc.vector.reciprocal(out=recip[:], in_=o_ps[:, :, :, DH:DH + 1])
            o_sb = sb_b.tile([T, 2, NH, DH], BF, tag="o_sb")
            nc.vector.tensor_tensor(
                out=o_sb[:], in0=o_ps[:, :, :, 0:DH],
                in1=recip[:].to_broadcast([128, 2, NH, DH]), op=ALU.mult,
            )

            oT_ps = psb.tile([DC, 2, T], BF, tag="bp")
            for bb in range(2):
                nc.tensor.transpose(
                    out=oT_ps[:, bb, :],
                    in_=o_sb[:, bb, :, :].rearrange("t h e -> t (h e)"),
                    identity=ident[:],
                )
            oT_sb = sb_b.tile([DC, 2, T], BF, tag="oT")
            nc.vector.tensor_copy(
                out=oT_sb[:].rearrange("p b t -> p (b t)"),
                in_=oT_ps[:].rearrange("p b t -> p (b t)"),
            )

            att_ps = psb.tile([T, 2, D], F32, tag="bp")
            for bb in range(2):
                nc.tensor.matmul(
                    att_ps[:, bb, :], oT_sb[:, bb, :], wo[:],
                    start=True, stop=True, skip_group_check=True,
                )

            nc.vector.tensor_tensor(
                out=x1_all[:, 2 * p:2 * p + 2, :],
                in0=x_t[:, 2 * p:2 * p + 2, :], in1=att_ps[:], op=ALU.add,
            )
            if flags["bo"]:
                bo4 = bass.AP(
                    tensor=bo_rep.tensor, offset=bo_rep[:].offset,
                    ap=[bo_rep[:].ap[0], [0, 2], bo_rep[:].ap[1]],
                )
                nc.vector.tensor_tensor(
                    out=x1_all[:, 2 * p:2 * p + 2, :],
                    in0=x1_all[:, 2 * p:2 * p + 2, :], in1=bo4, op=ALU.add,
                )
        return x1_all

    def phase_C(g, x1_all):
        """LN2 + feed-forward + residual + store, in pairs of batches."""
        aggr2, rstd2 = ln_stats(x1_all, "2")

        for p in range(G // 2):
            hh2 = sb_b.tile([T, 2, D], BF, tag="hh2")
            for bb in range(2):
                b = 2 * p + bb
                ln_apply(hh2[:, bb, :], x1_all[:, b, :], aggr2, rstd2, b)

            h2T_ps = psf.tile([D, 2, T], BF, tag="fp")
            for bb in range(2):
                nc.tensor.transpose(
                    out=h2T_ps[:, bb, :], in_=hh2[:, bb, :], identity=ident[:],
                )
            h2T = sb_b.tile([D, 2, T], BF, tag="h2T")
            nc.vector.tensor_copy(
                out=h2T[:].rearrange("d b t -> d (b t)"),
                in_=h2T_ps[:].rearrange("d b t -> d (b t)"),
            )

            # ff1 over the batch pair: one stationary load per w1 chunk,
            # N=256 moving; two chunks share one PSUM bank
            r_sb = sb_b.tile([128, 2, 4, T], BF, tag="r_sb")
            for cc in range(2):
                ff1_ps = psf.tile([128, 2, 2, T], F32, tag="fp")
                for c2 in range(2):
                    c = 2 * cc + c2
                    nc.tensor.matmul(
                        ff1_ps[:, c2, :, :].rearrange("p b t -> p (b t)"),
                        w1[:, 128 * c:128 * (c + 1)],
                        h2T[:].rearrange("d b t -> d (b t)"),
                        start=True, stop=True, skip_group_check=True,
                    )
                if flags["b1"]:
                    for c2 in range(2):
                        c = 2 * cc + c2
                        for bb in range(2):
                            nc.scalar.activation(
                                out=r_sb[:, bb, c, :], in_=ff1_ps[:, c2, bb, :],
                                func=AF.Relu, bias=b1e[:, c:c + 1], scale=1.0,
                            )
                else:
                    nc.scalar.activation(
                        out=r_sb[:, :, 2 * cc:2 * cc + 2, :].rearrange(
                            "p b c t -> p c b t"
                        ),
                        in_=ff1_ps[:],
                        func=AF.Relu,
                    )

            ff2_ps = psf.tile([T, 2, D], F32, tag="fp")
            for bb in range(2):
                for c in range(4):
                    nc.tensor.matmul(
                        ff2_ps[:, bb, :], r_sb[:, bb, c, :], w2[:, c, :],
                        start=(c == 0), stop=(c == 3), skip_group_check=True,
                    )

            out_sb = sb_b.tile([T, 2, D], F32, tag="out_sb")
            nc.vector.tensor_tensor(
                out=out_sb[:], in0=x1_all[:, 2 * p:2 * p + 2, :], in1=ff2_ps[:],
                op=ALU.add,
            )
            if flags["b2"]:
                b24 = bass.AP(
                    tensor=b2_rep.tensor, offset=b2_rep[:].offset,
                    ap=[b2_rep[:].ap[0], [0, 2], b2_rep[:].ap[1]],
                )
                nc.vector.tensor_tensor(
                    out=out_sb[:], in0=out_sb[:], in1=b24, op=ALU.add,
                )
            a = g * G + 2 * p
            nc.gpsimd.dma_start(
                out=y_ap[a:a + 2].rearrange("b t d -> t b d"), in_=out_sb[:]
            )

    # software pipeline: B(g) | A(g+1) | C(g)
    x_cur = load_x(glist[0])
    A_cur = phase_A(x_cur)
    for gi, g in enumerate(glist):
        more = gi + 1 < len(glist)
        if more:
            x_nxt = load_x(glist[gi + 1])
        x1 = phase_B(x_cur, *A_cur)
        if more:
            A_nxt = phase_A(x_nxt)
        phase_C(g, x1)
        if more:
            x_cur, A_cur = x_nxt, A_nxt


def build_program(weights, flags, bpc=BPC):
    nc = bacc.Bacc("TRN2", target_bir_lowering=False, debug=False)
    aps = {}
    aps["x"] = nc.dram_tensor("x", [bpc, T, D], F32, kind="ExternalInput").ap()
    aps["y"] = nc.dram_tensor("y", [bpc, T, D], F32, kind="ExternalOutput").ap()
    for name, arr in weights.items():
        dt = F32 if arr.dtype == np.float32 else BF
        aps[name] = nc.dram_tensor(name, list(arr.shape), dt, kind="ExternalInput").ap()
    with tile.TileContext(nc) as tc:
        with ExitStack() as ctx:
            _emit(ctx, tc, aps, flags, bpc)
    nc.compile()
    return nc


_CACHE = {}


def _get_program_and_maps(x, args):
    x = np.asarray(x, np.float32)
    weights, flags = _prep_weights(*args)
    key = tuple(sorted(flags.items()))
    if key not in _CACHE:
        _CACHE[key] = build_program(weights, flags)
    nc = _CACHE[key]
    in_maps = []
    for c in range(NCORES):
        m = {"x": np.ascontiguousarray(x[c * BPC:(c + 1) * BPC])}
        m.update(weights)
        in_maps.append(m)
    return nc, in_maps


def kernel(x, Wq, Wk, Wv, Wo, bo, W1, b1, W2, b2, g1, beta1, g2, beta2):
    nc, in_maps = _get_program_and_maps(
        x, (Wq, Wk, Wv, Wo, bo, W1, b1, W2, b2, g1, beta1, g2, beta2)
    )
    res = run_bass_kernel_spmd(nc, in_maps, list(range(NCORES)))
    out = np.concatenate([res.results[c]["y"] for c in range(NCORES)], axis=0)
    return out.astype(np.float32)


def run_traced(inputs):
    """Profiled run; returns BassKernelResults with exec_time_ns if available."""
    args = tuple(
        inputs[k]
        for k in ("Wq", "Wk", "Wv", "Wo", "bo", "W1", "b1", "W2", "b2",
                  "g1", "beta1", "g2", "beta2")
    )
    nc, in_maps = _get_program_and_maps(inputs["x"], args)
    return run_bass_kernel_spmd(nc, in_maps, list(range(NCORES)), trace=True)
